# revision 1
# baseline (speedup 1.0000x reference)
"""CrossConsensus kernel for 8 Trainium2 NeuronCores.

Sharding: data-parallel over B*L rows. Core c handles batch b=c//4,
target rows [ (c%4)*512, (c%4+1)*512 ).  All computation is row-local
(edge_i = repeat(arange(L), 8) means each edge scatters back to its own
source row), so there are no collectives; each core needs its target
row-chunk plus the full context of its batch.
"""

import math

import numpy as np

import concourse.bass as bass
import concourse.bacc as bacc
import concourse.tile as tile
from concourse import mybir
from concourse.bass_utils import run_bass_kernel_spmd
from concourse.masks import make_identity

F32 = mybir.dt.float32
BF16 = mybir.dt.bfloat16
U32 = mybir.dt.uint32
AX = mybir.AxisListType
ALU = mybir.AluOpType
ACTF = mybir.ActivationFunctionType

# problem constants (hardcoded per the harness contract)
B, L, K, D = 2, 2048, 2048, 512
H, R, WWIN, T, EH = 8, 8, 8, 2, 16
HD = D // H            # 64
LC = L * B // 8        # 512 rows per core
NT = LC // 128         # 4 l-tiles per core
KT = K // 128          # 16 k-tiles
CROW = D + 2 * EH      # 544: gather-table row [v(512) | ca(16) | cl(16)]
R9 = R + 1             # rank slots incl. alpha slot
TWO_PI = 2.0 * math.pi


def build_program():
    nc = bacc.Bacc()

    # ---------------- external I/O ----------------
    tT = nc.dram_tensor("tT", [D, LC], F32, kind="ExternalInput")        # target^T
    cT = nc.dram_tensor("cT", [D, K], F32, kind="ExternalInput")         # context^T
    Wt_d = nc.dram_tensor("Wt", [D, D], F32, kind="ExternalInput")
    WtR_d = nc.dram_tensor("WtR", [D, D], F32, kind="ExternalInput")     # rotate_half-folded
    Wc_d = nc.dram_tensor("Wc", [D, D], F32, kind="ExternalInput")
    Wo_d = nc.dram_tensor("Wo", [D, D], F32, kind="ExternalInput")
    bt_d = nc.dram_tensor("bt", [1, D], F32, kind="ExternalInput")
    btR_d = nc.dram_tensor("btR", [1, D], F32, kind="ExternalInput")
    bc_d = nc.dram_tensor("bc", [1, D], F32, kind="ExternalInput")
    bo_d = nc.dram_tensor("bo", [1, D], F32, kind="ExternalInput")
    Wtr3_d = nc.dram_tensor("Wtr3", [D, 48], F32, kind="ExternalInput")  # [Ws1t|Wa1t|Wl1t]
    Ws1c_d = nc.dram_tensor("Ws1c", [D, EH], F32, kind="ExternalInput")
    Wacl_d = nc.dram_tensor("Wacl", [D, 32], F32, kind="ExternalInput")  # [Wa1c|Wl1c]
    bs1_d = nc.dram_tensor("bs1", [1, EH], F32, kind="ExternalInput")
    bacl_d = nc.dram_tensor("bacl", [1, 32], F32, kind="ExternalInput")  # [ba1|bl1]
    Ws2bd_d = nc.dram_tensor("Ws2bd", [128, 8], F32, kind="ExternalInput")
    Wa2_d = nc.dram_tensor("Wa2", [1, EH], F32, kind="ExternalInput")
    ba2_d = nc.dram_tensor("ba2", [1, 1], F32, kind="ExternalInput")
    Wl2_d = nc.dram_tensor("Wl2", [128, H * R * HD], BF16, kind="ExternalInput")
    stp_d = nc.dram_tensor("stp", [128, T * NT], F32, kind="ExternalInput")
    lcol_d = nc.dram_tensor("lcol", [128, NT], F32, kind="ExternalInput")
    invf_d = nc.dram_tensor("invf", [1, HD // 2], F32, kind="ExternalInput")
    y_d = nc.dram_tensor("y", [LC, D], F32, kind="ExternalOutput")

    # internal DRAM gather table
    Tctx = nc.dram_tensor("Tctx", [K, CROW], BF16)

    # ---------------- persistent SBUF (static allocs, before pools) ----------
    ident = nc.alloc_sbuf_tensor("ident", [128, 128], F32).ap()
    ones1 = nc.alloc_sbuf_tensor("ones1", [1, 512], F32).ap()
    u_sb = [nc.alloc_sbuf_tensor(f"u{i}", [128, D], F32).ap() for i in range(NT)]
    uR_sb = [nc.alloc_sbuf_tensor(f"uR{i}", [128, D], F32).ap() for i in range(NT)]
    trio = [nc.alloc_sbuf_tensor(f"trio{i}", [128, 48], F32).ap() for i in range(NT)]
    Wl2_sb = nc.alloc_sbuf_tensor("Wl2sb", [128, H * R * HD], BF16).ap()
    cpTrep = nc.alloc_sbuf_tensor("cpTrep", [128, K], F32).ap()
    tpbT = nc.alloc_sbuf_tensor("tpbT", [128, NT * 16], F32).ap()
    invf_sb = nc.alloc_sbuf_tensor("invfsb", [128, HD // 2], F32).ap()
    wa2_sb = nc.alloc_sbuf_tensor("wa2sb", [128, EH], F32).ap()
    ba2_sb = nc.alloc_sbuf_tensor("ba2sb", [128, 1], F32).ap()
    stp_sb = nc.alloc_sbuf_tensor("stpsb", [128, T * NT], F32).ap()
    stpn_sb = nc.alloc_sbuf_tensor("stpnsb", [128, T * NT], F32).ap()
    lcol_sb = nc.alloc_sbuf_tensor("lcolsb", [128, NT], F32).ap()
    bs1_sb = nc.alloc_sbuf_tensor("bs1sb", [1, EH], F32).ap()
    bacl_sb = nc.alloc_sbuf_tensor("baclsb", [1, 32], F32).ap()
    bt_sb = nc.alloc_sbuf_tensor("btsb", [1, D], F32).ap()
    btR_sb = nc.alloc_sbuf_tensor("btRsb", [1, D], F32).ap()
    bc_sb = nc.alloc_sbuf_tensor("bcsb", [1, D], F32).ap()
    bo_sb = nc.alloc_sbuf_tensor("bosb", [1, D], F32).ap()
    Ws2bd_sb = nc.alloc_sbuf_tensor("ws2bdsb", [128, 8], F32).ap()
    Wtr3_sb = nc.alloc_sbuf_tensor("wtr3sb", [128, 4 * 48], F32).ap()
    Wacl_sb = nc.alloc_sbuf_tensor("waclsb", [128, 4 * 32], F32).ap()
    halfpi = nc.alloc_sbuf_tensor("halfpi", [128, 1], F32).ap()
    onec = nc.alloc_sbuf_tensor("onec", [128, 1], F32).ap()
    onesb = nc.alloc_sbuf_tensor("onesb", [1, 256], BF16).ap()

    with tile.TileContext(nc) as tc:
        with (
            tc.tile_pool(name="ld", bufs=3) as ldp,             # small staging tiles
            tc.tile_pool(name="gbp", bufs=1) as gbp,            # gather block
            tc.tile_pool(name="lamp", bufs=2) as lamp,          # Lam
            tc.tile_pool(name="prodp", bufs=2) as prodp,        # einsum products
            tc.tile_pool(name="med", bufs=2) as medp,
            tc.tile_pool(name="sml", bufs=2) as smlp,
            tc.tile_pool(name="wp", bufs=1) as wp,
            tc.tile_pool(name="ps", bufs=2, space="PSUM") as psp,
            tc.tile_pool(name="ps4", bufs=4, space="PSUM") as ps4p,
        ):
            # ---------- constants ----------
            make_identity(nc, ident)
            nc.vector.memset(ones1, 1.0)
            nc.vector.memset(halfpi, math.pi / 2)
            nc.vector.memset(onec, 1.0)
            nc.vector.memset(onesb, 1.0)
            nc.sync.dma_start(out=invf_sb, in_=invf_d[:].partition_broadcast(128))
            nc.sync.dma_start(out=wa2_sb, in_=Wa2_d[:].partition_broadcast(128))
            nc.sync.dma_start(out=ba2_sb, in_=ba2_d[:].partition_broadcast(128))
            nc.sync.dma_start(out=lcol_sb, in_=lcol_d[:])
            nc.sync.dma_start(out=bs1_sb, in_=bs1_d[:])
            nc.sync.dma_start(out=bacl_sb, in_=bacl_d[:])
            nc.sync.dma_start(out=bt_sb, in_=bt_d[:])
            nc.sync.dma_start(out=btR_sb, in_=btR_d[:])
            nc.sync.dma_start(out=bc_sb, in_=bc_d[:])
            nc.sync.dma_start(out=bo_sb, in_=bo_d[:])
            nc.sync.dma_start(out=Ws2bd_sb, in_=Ws2bd_d[:])
            nc.sync.dma_start(out=Wl2_sb, in_=Wl2_d[:])
            for dc in range(4):
                sl = slice(dc * 128, (dc + 1) * 128)
                nc.sync.dma_start(out=Wtr3_sb[:, dc * 48:(dc + 1) * 48], in_=Wtr3_d[sl, :])
                nc.sync.dma_start(out=Wacl_sb[:, dc * 32:(dc + 1) * 32], in_=Wacl_d[sl, :])

            def load_w(dram):
                t = wp.tile([128, 4 * D], F32, tag="wrhs")
                for dc in range(4):
                    nc.sync.dma_start(out=t[:, dc * D:(dc + 1) * D],
                                      in_=dram[dc * 128:(dc + 1) * 128, :])
                return t

            def softplus(dst, src, bias_ap, tmp_pool, tmp_tag):
                """dst = softplus(src + bias) = relu(x) + ln(1+exp(-|x|)).
                No softplus HW table; composed from abs/exp/ln (one table set)."""
                shp = [src.shape[0], src.free_size()]
                a = tmp_pool.tile(shp, F32, tag=tmp_tag)
                if bias_ap is None:
                    nc.scalar.activation(a[:], src, ACTF.Abs)
                    nc.vector.tensor_scalar(dst, src, 0.0, scalar2=None, op0=ALU.max)
                else:
                    nc.scalar.activation(a[:], src, ACTF.Abs, bias=bias_ap)
                    nc.vector.tensor_scalar(dst, src, bias_ap, scalar2=0.0,
                                            op0=ALU.add, op1=ALU.max)
                nc.scalar.activation(a[:], a[:], ACTF.Exp, scale=-1.0)
                nc.scalar.activation(a[:], a[:], ACTF.Ln, bias=onec[:, 0:1])
                nc.vector.tensor_tensor(dst, dst, a[:], op=ALU.add)

            stp_raw = smlp.tile([128, T * NT], F32, tag="stpraw")
            nc.sync.dma_start(out=stp_raw[:], in_=stp_d[:])
            softplus(stp_sb, stp_raw[:], None, smlp, "sptmp")
            nc.vector.tensor_scalar_mul(stpn_sb, stp_sb, -1.0)

            # ---------- dense projections ----------
            def mm_rows(out_ap, lhsT_dram, tix, w_sb, ncol, bias_sb, evac="v"):
                """out[128 rows of tile tix, ncol] = lhsT_dram[:, tile].T @ W (+ bias)."""
                ps = psp.tile([128, 512], F32, space="PSUM", tag="mmps")
                have_bias = bias_sb is not None
                if have_bias:
                    nc.tensor.matmul(ps[:, :ncol], ones1[:1, :128],
                                     bias_sb[:1, :ncol], start=True, stop=False)
                for dc in range(4):
                    lh = ldp.tile([128, 128], F32, tag="lhst")
                    nc.sync.dma_start(
                        out=lh[:], in_=lhsT_dram[dc * 128:(dc + 1) * 128,
                                                 tix * 128:(tix + 1) * 128])
                    nc.tensor.matmul(ps[:, :ncol], lh[:],
                                     w_sb[:, dc * ncol:(dc + 1) * ncol],
                                     start=(not have_bias and dc == 0),
                                     stop=(dc == 3))
                if evac == "v":
                    nc.scalar.copy(out_ap, ps[:, :ncol])
                else:  # DRAM destination: stage through SBUF (DMA can't read PSUM)
                    stg = ldp.tile([128, 512], BF16, tag="stgb")
                    nc.scalar.copy(stg[:, :ncol], ps[:, :ncol])
                    nc.sync.dma_start(out=out_ap, in_=stg[:, :ncol])

            Wt_t = load_w(Wt_d)
            for lt in range(NT):
                mm_rows(u_sb[lt][:], tT, lt, Wt_t[:], D, bt_sb)
            WtR_t = load_w(WtR_d)
            for lt in range(NT):
                mm_rows(uR_sb[lt][:], tT, lt, WtR_t[:], D, btR_sb)
                mm_rows(trio[lt][:], tT, lt, Wtr3_sb, 48, None)

            Wc_t = load_w(Wc_d)
            for kt in range(KT):
                mm_rows(Tctx[kt * 128:(kt + 1) * 128, 0:D], cT, kt, Wc_t[:], D,
                        bc_sb, evac="dma")
                mm_rows(Tctx[kt * 128:(kt + 1) * 128, D:D + 32], cT, kt, Wacl_sb, 32,
                        bacl_sb, evac="dma")

            # tpbT: per-octet score bias columns, partition layout p = ls*16 + e
            for lt in range(NT):
                for oc in range(16):
                    nc.sync.dma_start(
                        out=tpbT[:, lt * 16 + oc:lt * 16 + oc + 1],
                        in_=trio[lt][oc * 8:(oc + 1) * 8, 0:EH])

            # cpT [16, K] = Ws1c.T @ context^T + bs1, then replicate 8x on partitions
            cpT_t = medp.tile([EH, K], F32, tag="cpTt")
            cpT = cpT_t[:]
            for nt4 in range(4):
                nsl = slice(nt4 * 512, (nt4 + 1) * 512)
                ps = psp.tile([128, 512], F32, space="PSUM", tag="mmps")
                nc.tensor.matmul(ps[:EH, :], bs1_sb[:1, :], ones1[:1, :512],
                                 start=True, stop=False)
                for dc in range(4):
                    lh = ldp.tile([128, EH], F32, tag="lhst16")
                    nc.sync.dma_start(out=lh[:],
                                      in_=Ws1c_d[dc * 128:(dc + 1) * 128, :])
                    rh = ldp.tile([128, 512], F32, tag="ctchunk")
                    nc.sync.dma_start(out=rh[:], in_=cT[dc * 128:(dc + 1) * 128, nsl])
                    nc.tensor.matmul(ps[:EH, :], lh[:], rh[:],
                                     start=False, stop=(dc == 3))
                nc.vector.tensor_copy(cpT[:, nsl], ps[:EH, :])
            for ls in range(8):
                nc.sync.dma_start(out=cpTrep[ls * 16:(ls + 1) * 16, :], in_=cpT[:, :])

            # ---------- per l-tile ----------
            for lt in range(NT):
                # ----- scores + top-8 -----
                scores = medp.tile([128, K], F32, tag="scores")
                for oc in range(16):
                    for hf in range(2):
                        g_sc = medp.tile([128, K // 2], F32, tag="gsc")
                        nc.scalar.activation(
                            g_sc[:], cpTrep[:, hf * 1024:(hf + 1) * 1024], ACTF.Gelu,
                            bias=tpbT[:, lt * 16 + oc:lt * 16 + oc + 1])
                        for nq in range(2):
                            col = hf * 1024 + nq * 512
                            pssc = psp.tile([8, 512], F32, space="PSUM", tag="small")
                            nc.tensor.matmul(pssc[:, :], Ws2bd_sb[:],
                                             g_sc[:, nq * 512:(nq + 1) * 512],
                                             start=True, stop=True)
                            sstg = medp.tile([8, 512], F32, tag="sstg")
                            nc.vector.tensor_copy(sstg[:], pssc[:, :])
                            nc.sync.dma_start(
                                out=scores[oc * 8:(oc + 1) * 8, col:col + 512],
                                in_=sstg[:])
                mx8 = smlp.tile([128, 8], F32, tag="mx8")
                idx = smlp.tile([128, 8], U32, tag="idx")
                nc.vector.max(out=mx8[:], in_=scores[:])
                nc.vector.max_index(out=idx[:], in_max=mx8[:], in_values=scores[:])

                # ----- gather context-side rows -----
                gb = gbp.tile([128, WWIN * CROW], BF16, tag="gb")
                gbv = gb[:].rearrange("p (w c) -> p w c", w=8)
                for w in range(WWIN):
                    nc.gpsimd.indirect_dma_start(
                        out=gb[:, w * CROW:(w + 1) * CROW],
                        out_offset=None,
                        in_=Tctx[:, :],
                        in_offset=bass.IndirectOffsetOnAxis(ap=idx[:, w:w + 1], axis=0),
                    )

                # ----- per-edge angles -----
                jf = smlp.tile([128, 8], F32, tag="jf")
                nc.vector.tensor_copy(jf[:], idx[:])
                delta = smlp.tile([128, 8], F32, tag="delta")
                nc.vector.tensor_scalar(delta[:], jf[:], lcol_sb[:, lt:lt + 1],
                                        scalar2=None, op0=ALU.subtract)
                ang = medp.tile([128, 8 * 32], F32, tag="ang")
                nc.vector.tensor_tensor(
                    out=ang[:].rearrange("p (w f) -> p w f", w=8),
                    in0=delta[:].unsqueeze(2).to_broadcast((128, 8, 32)),
                    in1=invf_sb[:].unsqueeze(1).to_broadcast((128, 8, 32)),
                    op=ALU.mult)
                # range-reduce to [-pi, pi]: x - 2pi*round(x/2pi), round via
                # the +/- 1.5*2^23 magic-number trick (no mod/floor on DVE ISA)
                MAGIC = 1.5 * 2.0 ** 23
                angt = medp.tile([128, 8 * 32], F32, tag="angt")
                nc.vector.tensor_scalar_mul(angt[:], ang[:], 1.0 / TWO_PI)
                angr = medp.tile([128, 8 * 32], F32, tag="angr")
                nc.vector.tensor_scalar(angr[:], angt[:], MAGIC, scalar2=MAGIC,
                                        op0=ALU.add, op1=ALU.subtract)
                nc.vector.tensor_sub(angt[:], angt[:], angr[:])
                nc.vector.tensor_scalar_mul(ang[:], angt[:], TWO_PI)
                cosb = medp.tile([128, 8 * 32], F32, tag="cosb")
                sinb = medp.tile([128, 8 * 32], F32, tag="sinb")
                nc.scalar.activation(sinb[:], ang[:], ACTF.Sin, scale=-1.0)
                nc.vector.tensor_scalar_mul(angr[:], ang[:], -1.0)
                nc.vector.tensor_max(angt[:], ang[:], angr[:])
                nc.scalar.activation(cosb[:], angt[:], ACTF.Sin, scale=-1.0,
                                     bias=halfpi[:, 0:1])

                # ----- alphas = softplus(gelu(ta+ca) @ Wa2 + ba2) -----
                ha = smlp.tile([128, 8 * EH], F32, tag="ha")
                nc.vector.tensor_tensor(
                    out=ha[:].rearrange("p (w c) -> p w c", w=8),
                    in0=trio[lt][:, 16:32].unsqueeze(1).to_broadcast((128, 8, EH)),
                    in1=gbv[:, :, D:D + EH],
                    op=ALU.add)
                nc.scalar.activation(ha[:], ha[:], ACTF.Gelu)
                haw = smlp.tile([128, 8 * EH], F32, tag="haw")
                nc.vector.tensor_tensor(
                    out=haw[:].rearrange("p (w c) -> p w c", w=8),
                    in0=ha[:].rearrange("p (w c) -> p w c", w=8),
                    in1=wa2_sb[:].unsqueeze(1).to_broadcast((128, 8, EH)),
                    op=ALU.mult)
                alphas = smlp.tile([128, 8], F32, tag="alphas")
                nc.vector.tensor_reduce(alphas[:], haw[:].rearrange(
                    "p (w c) -> p w c", w=8), axis=AX.X, op=ALU.add)
                softplus(alphas[:], alphas[:], ba2_sb[:, 0:1], smlp, "sptmp")

                # ----- g = gelu(tl + cl) and per-w transposes -----
                gmat = smlp.tile([128, 8 * EH], F32, tag="gmat")
                nc.vector.tensor_tensor(
                    out=gmat[:].rearrange("p (w c) -> p w c", w=8),
                    in0=trio[lt][:, 32:48].unsqueeze(1).to_broadcast((128, 8, EH)),
                    in1=gbv[:, :, D + EH:D + 2 * EH],
                    op=ALU.add)
                nc.scalar.activation(gmat[:], gmat[:], ACTF.Gelu)
                gT4 = gbp.tile([128, 2 * 128], BF16, tag="gT4")  # 2 quads side by side
                nc.vector.memset(gT4[:], 0.0)
                for s4 in range(4):  # bias row (constant 1) for the bl2 fold
                    nc.sync.dma_start(out=gT4[32 * s4 + EH:32 * s4 + EH + 1, :],
                                      in_=onesb[:1, 0:256])
                for w in range(WWIN):
                    q, s = w // 4, w % 4
                    pst = psp.tile([EH, 128], F32, space="PSUM", tag="small")
                    nc.tensor.transpose(
                        out=pst[:, :],
                        in_=gmat[:].rearrange("p (w c) -> p w c", w=8)[:, w, :],
                        identity=ident)
                    nc.vector.tensor_copy(
                        gT4[32 * s:32 * s + EH, q * 128:(q + 1) * 128], pst[:, :])

                # ----- per-head loop -----
                for h in range(H):
                    # Lam layout: (w, r9, d) bf16, contiguous
                    Lam = lamp.tile([128, WWIN * R9 * HD], BF16, tag="lam")
                    for w in range(WWIN):
                        q, s = w // 4, w % 4
                        psl = ps4p.tile([128, 512], F32, space="PSUM", tag="lamps")
                        nc.tensor.matmul(
                            psl[:, :], gT4[32 * s:32 * s + 32, q * 128:(q + 1) * 128],
                            Wl2_sb[32 * s:32 * s + 32, h * R * HD:(h + 1) * R * HD],
                            start=True, stop=True, tile_position=(32 * s, 0))
                        nc.scalar.copy(
                            Lam[:, w * R9 * HD:w * R9 * HD + R * HD], psl[:, :])
                    lam4 = Lam[:].rearrange("p (w r d) -> p w r d", w=8, r=R9)
                    # squared row norms -> scale 1/max(norm,1e-12)^2 (square on ACT)
                    n2 = smlp.tile([128, WWIN * R], F32, tag="n2")
                    for w in range(WWIN):
                        sqw = medp.tile([128, R * HD], F32, tag="sqw")
                        nc.scalar.activation(sqw[:], lam4[:, w, 0:R, :], ACTF.Square)
                        nc.vector.tensor_reduce(
                            n2[:].rearrange("p (w r) -> p w r", w=8)[:, w, :],
                            sqw[:].rearrange("p (r d) -> p r d", r=R),
                            axis=AX.X, op=ALU.add)
                    nrm = smlp.tile([128, WWIN * R], F32, tag="nrm")
                    nc.vector.tensor_scalar_max(nrm[:], n2[:], 1e-24)
                    rec9 = smlp.tile([128, WWIN * R9], F32, tag="rec9")
                    nc.vector.memset(rec9[:], 0.0)
                    nc.vector.reciprocal(
                        rec9[:].rearrange("p (w r) -> p w r", w=8, r=R9)[:, :, 0:R],
                        nrm[:].rearrange("p (w r) -> p w r", w=8))

                    usl = u_sb[lt][:, h * HD:(h + 1) * HD]
                    uRsl = uR_sb[lt][:, h * HD:(h + 1) * HD]
                    for t in range(T):
                        stc = slice(t * NT + lt, t * NT + lt + 1)
                        # diff = u_i*cos + uR_i*sin - v_j     [128, (w,d)]
                        diff = medp.tile([128, WWIN * HD], BF16, tag="diff")
                        d3 = diff[:].rearrange("p (w d) -> p w d", w=8)
                        t0 = medp.tile([128, WWIN * HD], BF16, tag="t0")
                        nc.vector.tensor_tensor(
                            out=t0[:].rearrange("p (w a b) -> p w a b", w=8, a=2),
                            in0=usl.rearrange("p (a b) -> p a b", a=2)
                                .unsqueeze(1).to_broadcast((128, 8, 2, 32)),
                            in1=cosb[:].rearrange("p (w f) -> p w f", w=8)
                                .unsqueeze(2).to_broadcast((128, 8, 2, 32)),
                            op=ALU.mult)
                        t1 = medp.tile([128, WWIN * HD], BF16, tag="t0")
                        nc.vector.tensor_tensor(
                            out=t1[:].rearrange("p (w a b) -> p w a b", w=8, a=2),
                            in0=uRsl.rearrange("p (a b) -> p a b", a=2)
                                .unsqueeze(1).to_broadcast((128, 8, 2, 32)),
                            in1=sinb[:].rearrange("p (w f) -> p w f", w=8)
                                .unsqueeze(2).to_broadcast((128, 8, 2, 32)),
                            op=ALU.mult)
                        nc.vector.tensor_tensor(out=t0[:], in0=t0[:], in1=t1[:],
                                                op=ALU.add)
                        nc.vector.tensor_tensor(
                            out=d3,
                            in0=t0[:].rearrange("p (w d) -> p w d", w=8),
                            in1=gbv[:, :, h * HD:(h + 1) * HD],
                            op=ALU.subtract)
                        # alpha slot: Lam[:, :, 8, :] = diff
                        nc.vector.tensor_copy(lam4[:, :, R:R9, :].squeeze(2), d3)
                        # einsum1: Ld[w,r] = sum_d Lam*diff
                        prod = prodp.tile([128, WWIN * R9 * HD], BF16, tag="prodb")
                        nc.vector.tensor_tensor(
                            out=prod[:].rearrange("p (w r d) -> p w r d", w=8, r=R9),
                            in0=lam4,
                            in1=d3.unsqueeze(2).to_broadcast((128, 8, R9, HD)),
                            op=ALU.mult)
                        ld = smlp.tile([128, WWIN * R9], F32, tag="ld")
                        nc.vector.tensor_reduce(
                            ld[:].rearrange("p (w r) -> p w r", w=8),
                            prod[:].rearrange("p (w r d) -> p w r d", w=8, r=R9),
                            axis=AX.X, op=ALU.add)
                        ld2 = smlp.tile([128, WWIN * R9], F32, tag="ld2")
                        nc.vector.tensor_tensor(ld2[:], ld[:], rec9[:], op=ALU.mult)
                        nc.vector.tensor_copy(
                            ld2[:].rearrange("p (w r) -> p w r", w=8)[:, :, R:R9]
                            .squeeze(2), alphas[:])
                        # einsum2 (+alpha*diff via slot 8): res[w,d] = sum_r9 Ld2*Lam
                        # einsum2 products on gpsimd, split in w-halves so the
                        # DVE reduce of half 1 overlaps gpsimd's half 2
                        prod2 = prodp.tile([128, WWIN * R9 * HD], BF16, tag="prodb2")
                        p2v = prod2[:].rearrange("p (w d r) -> p w d r", w=8, d=HD)
                        lamT = lam4.transpose([0, 1, 3, 2])
                        ld2v = ld2[:].rearrange("p (w r) -> p w r", w=8)\
                            .unsqueeze(2).to_broadcast((128, 8, HD, R9))
                        res = medp.tile([128, WWIN * HD], F32, tag="res")
                        resv = res[:].rearrange("p (w d) -> p w d", w=8)
                        for hw_ in range(2):
                            wsl = slice(hw_ * 4, hw_ * 4 + 4)
                            nc.gpsimd.tensor_tensor(
                                out=p2v[:, wsl], in0=lamT[:, wsl],
                                in1=ld2v[:, wsl], op=ALU.mult)
                            nc.vector.tensor_reduce(
                                resv[:, wsl], p2v[:, wsl], axis=AX.X, op=ALU.add)
                        # scatter: sum over w
                        rsum = smlp.tile([128, HD], F32, tag="rsum")
                        nc.vector.tensor_reduce(
                            rsum[:],
                            res[:].rearrange("p (w d) -> p d w", w=8),
                            axis=AX.X, op=ALU.add)
                        # u -= step*rsum ; uR via rotate_half identity
                        nc.vector.scalar_tensor_tensor(
                            out=usl, in0=rsum[:], scalar=stpn_sb[:, stc],
                            in1=usl, op0=ALU.mult, op1=ALU.add)
                        nc.vector.scalar_tensor_tensor(
                            out=uRsl[:, 0:32], in0=rsum[:, 32:64],
                            scalar=stp_sb[:, stc], in1=uRsl[:, 0:32],
                            op0=ALU.mult, op1=ALU.add)
                        nc.vector.scalar_tensor_tensor(
                            out=uRsl[:, 32:64], in0=rsum[:, 0:32],
                            scalar=stpn_sb[:, stc], in1=uRsl[:, 32:64],
                            op0=ALU.mult, op1=ALU.add)

            # ---------- output projection: y = u @ Wo + bo ----------
            Wo_t = load_w(Wo_d)
            for lt in range(NT):
                psy = psp.tile([128, 512], F32, space="PSUM", tag="mmps")
                nc.tensor.matmul(psy[:, :], ones1[:1, :128], bo_sb[:1, :],
                                 start=True, stop=False)
                for dc in range(4):
                    pst = psp.tile([128, 128], F32, space="PSUM", tag="small")
                    nc.tensor.transpose(
                        out=pst[:, :], in_=u_sb[lt][:, dc * 128:(dc + 1) * 128],
                        identity=ident)
                    uT = ldp.tile([128, 128], F32, tag="uT")
                    nc.vector.tensor_copy(uT[:], pst[:, :])
                    nc.tensor.matmul(psy[:, :], uT[:], Wo_t[:, dc * D:(dc + 1) * D],
                                     start=False, stop=(dc == 3))
                ystg = ldp.tile([128, 512], F32, tag="stg")
                nc.vector.tensor_copy(ystg[:], psy[:, :])
                nc.sync.dma_start(out=y_d[lt * 128:(lt + 1) * 128, :], in_=ystg[:])

    nc.finalize()
    return nc


def _rot_cols(Wm):
    """Fold rotate_half into output columns: out cols = [-cols(h, hi), cols(h, lo)]."""
    W4 = Wm.reshape(-1, H, 2, HD // 2)
    out = np.concatenate([-W4[:, :, 1], W4[:, :, 0]], axis=2)
    return np.ascontiguousarray(out.reshape(Wm.shape))


def make_in_maps(inputs):
    """Host-side prep: slice/transpose inputs into the 8 per-core input maps."""
    target = np.asarray(inputs["target"], np.float32)
    context = np.asarray(inputs["context"], np.float32)
    Wt = np.asarray(inputs["Wt"], np.float32)
    bt = np.asarray(inputs["bt"], np.float32)
    Wc = np.asarray(inputs["Wc"], np.float32)
    bc = np.asarray(inputs["bc"], np.float32)
    Ws1 = np.asarray(inputs["Ws1"], np.float32)
    bs1 = np.asarray(inputs["bs1"], np.float32)
    Ws2 = np.asarray(inputs["Ws2"], np.float32)
    Wa1 = np.asarray(inputs["Wa1"], np.float32)
    ba1 = np.asarray(inputs["ba1"], np.float32)
    Wa2 = np.asarray(inputs["Wa2"], np.float32)
    ba2 = np.asarray(inputs["ba2"], np.float32)
    Wl1 = np.asarray(inputs["Wl1"], np.float32)
    bl1 = np.asarray(inputs["bl1"], np.float32)
    Wl2 = np.asarray(inputs["Wl2"], np.float32)
    bl2 = np.asarray(inputs["bl2"], np.float32)
    step_sizes = np.asarray(inputs["step_sizes"], np.float32)
    Wo = np.asarray(inputs["Wo"], np.float32)
    bo = np.asarray(inputs["bo"], np.float32)


    import ml_dtypes
    Ws2bd = np.zeros((128, 8), np.float32)
    for ls in range(8):
        Ws2bd[ls * 16:(ls + 1) * 16, ls] = Ws2[:, 0]
    Wl2s = np.zeros((128, H * R * HD), np.float32)
    for s in range(4):
        Wl2s[32 * s:32 * s + EH, :] = Wl2
        Wl2s[32 * s + EH, :] = bl2
    Wl2s = Wl2s.astype(ml_dtypes.bfloat16)
    invf = (1.0 / (10000.0 ** (np.arange(0, HD, 2, dtype=np.float32) / HD)))[None, :]

    common = dict(
        Wt=Wt, WtR=_rot_cols(Wt), Wc=Wc, Wo=Wo,
        bt=bt[None, :], btR=_rot_cols(bt[None, :]), bc=bc[None, :], bo=bo[None, :],
        Wtr3=np.ascontiguousarray(np.concatenate([Ws1[:D], Wa1[:D], Wl1[:D]], axis=1)),
        Ws1c=np.ascontiguousarray(Ws1[D:]),
        Wacl=np.ascontiguousarray(np.concatenate([Wa1[D:], Wl1[D:]], axis=1)),
        bs1=bs1[None, :],
        bacl=np.concatenate([ba1, bl1])[None, :],
        Ws2bd=Ws2bd, Wa2=np.ascontiguousarray(Wa2.T),
        ba2=np.asarray(ba2, np.float32).reshape(1, 1), Wl2=Wl2s,
        invf=np.ascontiguousarray(invf, np.float32),
    )

    in_maps = []
    for c in range(8):
        b, rc = c // 4, c % 4
        rows = slice(rc * LC, (rc + 1) * LC)
        stp = np.ascontiguousarray(
            step_sizes[:, rows].reshape(T, NT, 128).transpose(2, 0, 1)
            .reshape(128, T * NT))
        lcol = np.ascontiguousarray(
            (rc * LC + np.arange(LC, dtype=np.float32)).reshape(NT, 128).T)
        m = dict(common)
        m.update(
            tT=np.ascontiguousarray(target[b, rows].T),
            cT=np.ascontiguousarray(context[b].T),
            stp=stp, lcol=lcol,
        )
        in_maps.append(m)
    return in_maps


_NC_CACHE = {}


def kernel(**inputs):
    if "nc" not in _NC_CACHE:
        _NC_CACHE["nc"] = build_program()
    nc = _NC_CACHE["nc"]
    in_maps = make_in_maps(inputs)
    res = run_bass_kernel_spmd(nc, in_maps, list(range(8)))
    out = np.empty((B, L, D), np.float32)
    for c in range(8):
        b, rc = c // 4, c % 4
        out[b, rc * LC:(rc + 1) * LC] = res.results[c]["y"]
    return out



# revision 29
# speedup vs baseline: 1.2054x; 1.2054x over previous
"""CrossConsensus kernel for 8 Trainium2 NeuronCores.

Sharding: data-parallel over B*L rows. Core c handles batch b=c//4,
target rows [ (c%4)*512, (c%4+1)*512 ).  All computation is row-local
(edge_i = repeat(arange(L), 8) means each edge scatters back to its own
source row), so there are no collectives; each core needs its target
row-chunk plus the full context of its batch.

v3 design notes:
- scores path is fp32 end to end (tp, cp, gelu, matmul): the top-8
  selection must match the fp32 reference on near-ties.  Everything
  downstream (u/v/Lam/einsums) runs in fp16, which DVE processes at its
  2x/4x packed rate and PE at the 16-bit column rate.
- scores accumulate into one [128,1024] psum tile per K-half via block
  one-hot stationaries, so scores land row-major with one ACT evac.
- per (l-tile, head): two fp16 copies of Lam: LamA (w,r,d) from the PE
  matmul for einsum1's reduce-over-d, and LamB (d,w,r9) built by one
  strided GpSimd copy so einsum2's reduce runs over (w,r9) at once and
  directly yields the per-row scatter sum.  LamB slot r=8 carries diff
  (written per iteration by GpSimd) so the alpha*diff term rides the
  same reduce (ld2 slot 8 = alpha).
- per-head pool tiles with bufs=2 so consecutive heads/l-tiles pipeline
  across PE (Lam matmuls) / ACT (evac, gelu) / GpSimd (LamB builds,
  gathers) / DVE (einsums).
"""

import math

import numpy as np

import concourse.bass as bass
import concourse.bacc as bacc
import concourse.tile as tile
from concourse import mybir
from concourse.bass_utils import run_bass_kernel_spmd
from concourse.masks import make_identity

F32 = mybir.dt.float32
F16 = mybir.dt.float16
U32 = mybir.dt.uint32
AX = mybir.AxisListType
ALU = mybir.AluOpType
ACTF = mybir.ActivationFunctionType

# problem constants (hardcoded per the harness contract)
B, L, K, D = 2, 2048, 2048, 512
H, R, WWIN, T, EH = 8, 8, 8, 2, 16
HD = D // H            # 64
LC = L * B // 8        # 512 rows per core
NT = LC // 128         # 4 l-tiles per core
KT = K // 128          # 16 k-tiles
CROW = D + 2 * EH      # 544: gather-table row [v(512) | ca(16) | cl(16)]
R9 = R + 1             # rank slots incl. alpha slot
WRD = WWIN * R * HD    # 4096: LamA per head
DWR = HD * WWIN * R9   # 4608: LamB per head
TWO_PI = 2.0 * math.pi


def build_program():
    nc = bacc.Bacc()

    # ---------------- external I/O ----------------
    tT = nc.dram_tensor("tT", [D, LC], F16, kind="ExternalInput")        # target^T
    tT32 = nc.dram_tensor("tT32", [D, LC], F32, kind="ExternalInput")
    cT = nc.dram_tensor("cT", [D, K], F16, kind="ExternalInput")         # context^T
    cT32 = nc.dram_tensor("cT32", [D, K], F32, kind="ExternalInput")
    Wt_d = nc.dram_tensor("Wt", [D, D], F16, kind="ExternalInput")
    WtR_d = nc.dram_tensor("WtR", [D, D], F16, kind="ExternalInput")     # rotate_half-folded
    Wc_d = nc.dram_tensor("Wc", [D, D], F16, kind="ExternalInput")
    Wo_d = nc.dram_tensor("Wo", [D, D], F16, kind="ExternalInput")
    bt_d = nc.dram_tensor("bt", [1, D], F32, kind="ExternalInput")
    btR_d = nc.dram_tensor("btR", [1, D], F32, kind="ExternalInput")
    bc_d = nc.dram_tensor("bc", [1, D], F32, kind="ExternalInput")
    bo_d = nc.dram_tensor("bo", [1, D], F32, kind="ExternalInput")
    Wtr3_d = nc.dram_tensor("Wtr3", [D, 48], F32, kind="ExternalInput")  # [Ws1t|Wa1t|Wl1t]
    Ws1c_d = nc.dram_tensor("Ws1c", [D, EH], F32, kind="ExternalInput")
    Wacl_d = nc.dram_tensor("Wacl", [D, 32], F16, kind="ExternalInput")  # [Wa1c|Wl1c]
    bs1_d = nc.dram_tensor("bs1", [1, EH], F32, kind="ExternalInput")
    bacl_d = nc.dram_tensor("bacl", [1, 32], F32, kind="ExternalInput")  # [ba1|bl1]
    Ws2oc_d = nc.dram_tensor("Ws2oc", [128, 16 * 128], F32, kind="ExternalInput")
    Wa2_d = nc.dram_tensor("Wa2", [1, EH], F32, kind="ExternalInput")
    ba2_d = nc.dram_tensor("ba2", [1, 1], F32, kind="ExternalInput")
    Wl2_d = nc.dram_tensor("Wl2", [128, H * R * HD], F16, kind="ExternalInput")
    stp_d = nc.dram_tensor("stp", [128, T * NT], F32, kind="ExternalInput")
    lcol_d = nc.dram_tensor("lcol", [128, NT], F32, kind="ExternalInput")
    invf_d = nc.dram_tensor("invf", [1, HD], F32, kind="ExternalInput")  # dup halves
    y_d = nc.dram_tensor("y", [LC, D], F32, kind="ExternalOutput")

    # internal DRAM gather table
    Tctx = nc.dram_tensor("Tctx", [K, CROW], F16)

    # ---------------- persistent SBUF (static allocs, before pools) ----------
    ident = nc.alloc_sbuf_tensor("ident", [128, 128], F32).ap()
    ones1 = nc.alloc_sbuf_tensor("ones1", [1, 512], F32).ap()
    U2 = [nc.alloc_sbuf_tensor(f"u2_{i}", [128, 2 * D], F32).ap() for i in range(NT)]
    trio = [nc.alloc_sbuf_tensor(f"trio{i}", [128, 48], F32).ap() for i in range(NT)]
    Wl2_sb = nc.alloc_sbuf_tensor("Wl2sb", [128, H * R * HD], F16).ap()
    Ws2oc_sb = nc.alloc_sbuf_tensor("ws2oc", [128, 16 * 128], F32).ap()
    cpTrep = nc.alloc_sbuf_tensor("cpTrep", [128, K], F32).ap()
    tpbT = nc.alloc_sbuf_tensor("tpbT", [128, NT * 16], F32).ap()
    invf_sb = nc.alloc_sbuf_tensor("invfsb", [128, HD], F32).ap()
    wa2_sb = nc.alloc_sbuf_tensor("wa2sb", [128, EH], F32).ap()
    ba2_sb = nc.alloc_sbuf_tensor("ba2sb", [128, 1], F32).ap()
    stp_sb = nc.alloc_sbuf_tensor("stpsb", [128, T * NT], F32).ap()
    stpn_sb = nc.alloc_sbuf_tensor("stpnsb", [128, T * NT], F32).ap()
    lcol_sb = nc.alloc_sbuf_tensor("lcolsb", [128, NT], F32).ap()
    bs1_sb = nc.alloc_sbuf_tensor("bs1sb", [1, EH], F32).ap()
    bacl_sb = nc.alloc_sbuf_tensor("baclsb", [1, 32], F32).ap()
    bt_sb = nc.alloc_sbuf_tensor("btsb", [1, D], F32).ap()
    btR_sb = nc.alloc_sbuf_tensor("btRsb", [1, D], F32).ap()
    bc_sb = nc.alloc_sbuf_tensor("bcsb", [1, D], F32).ap()
    bo_sb = nc.alloc_sbuf_tensor("bosb", [1, D], F32).ap()
    halfpi = nc.alloc_sbuf_tensor("halfpi", [128, 1], F32).ap()
    onec = nc.alloc_sbuf_tensor("onec", [128, 1], F32).ap()
    onesf = nc.alloc_sbuf_tensor("onesf", [1, 256], F16).ap()

    with tile.TileContext(nc) as tc:
        with (
            tc.tile_pool(name="ld", bufs=2) as ldp,             # small staging tiles
            tc.tile_pool(name="glp", bufs=2) as glp,            # scores gelu tiles
            tc.tile_pool(name="scp", bufs=1) as scp,            # scores f32
            tc.tile_pool(name="gbp", bufs=2) as gbp,            # gather block
            tc.tile_pool(name="lamA", bufs=2) as lamAp,         # per-head [128,4096]
            tc.tile_pool(name="lamB", bufs=2) as lamBp,         # per-head [128,4608]
            tc.tile_pool(name="prod", bufs=2) as prodp,         # per-head einsum scratch
            tc.tile_pool(name="ang", bufs=1) as angp,           # angle scratch
            tc.tile_pool(name="scr", bufs=2) as scrp,           # diff-build scratch
            tc.tile_pool(name="med", bufs=2) as medp,
            tc.tile_pool(name="sml", bufs=2) as smlp,
            tc.tile_pool(name="wp", bufs=1) as wp,
            tc.tile_pool(name="ps", bufs=1, space="PSUM") as psp,          # misc [128,512]
            tc.tile_pool(name="pssc", bufs=1, space="PSUM") as psscp,      # scores [128,1024]
            tc.tile_pool(name="psla", bufs=2, space="PSUM") as pslap,      # LamA [128,1024]
        ):
            # ---------- constants ----------
            make_identity(nc, ident)
            nc.vector.memset(ones1, 1.0)
            nc.vector.memset(halfpi, math.pi / 2)
            nc.vector.memset(onec, 1.0)
            nc.vector.memset(onesf, 1.0)

            def softplus(dst, src, bias_ap, tmp_pool, tmp_tag):
                """dst = softplus(src + bias) = relu(x) + ln(1+exp(-|x|)).
                No softplus HW table; composed from abs/exp/ln (one table set)."""
                shp = [src.shape[0], src.free_size()]
                a = tmp_pool.tile(shp, F32, tag=tmp_tag)
                if bias_ap is None:
                    nc.scalar.activation(a[:], src, ACTF.Abs)
                    nc.vector.tensor_scalar(dst, src, 0.0, scalar2=None, op0=ALU.max)
                else:
                    nc.scalar.activation(a[:], src, ACTF.Abs, bias=bias_ap)
                    nc.vector.tensor_scalar(dst, src, bias_ap, scalar2=0.0,
                                            op0=ALU.add, op1=ALU.max)
                nc.scalar.activation(a[:], a[:], ACTF.Exp, scale=-1.0)
                nc.scalar.activation(a[:], a[:], ACTF.Ln, bias=onec[:, 0:1])
                nc.vector.tensor_tensor(dst, dst, a[:], op=ALU.add)

            nc.sync.dma_start(out=invf_sb, in_=invf_d[:].partition_broadcast(128))
            nc.sync.dma_start(out=wa2_sb, in_=Wa2_d[:].partition_broadcast(128))
            nc.sync.dma_start(out=ba2_sb, in_=ba2_d[:].partition_broadcast(128))
            nc.sync.dma_start(out=lcol_sb, in_=lcol_d[:])
            nc.sync.dma_start(out=bs1_sb, in_=bs1_d[:])
            nc.sync.dma_start(out=bacl_sb, in_=bacl_d[:])
            nc.sync.dma_start(out=bt_sb, in_=bt_d[:])
            nc.sync.dma_start(out=btR_sb, in_=btR_d[:])
            nc.sync.dma_start(out=bc_sb, in_=bc_d[:])
            nc.sync.dma_start(out=bo_sb, in_=bo_d[:])
            nc.sync.dma_start(out=Ws2oc_sb, in_=Ws2oc_d[:])
            nc.sync.dma_start(out=Wl2_sb, in_=Wl2_d[:])

            stp_raw = smlp.tile([128, T * NT], F32, tag="stpraw")
            nc.sync.dma_start(out=stp_raw[:], in_=stp_d[:])
            softplus(stp_sb, stp_raw[:], None, smlp, "sptmp")
            nc.vector.tensor_scalar_mul(stpn_sb, stp_sb, -1.0)

            # ---------- dense projections ----------
            def load_w(dram, dt=F16):
                t = wp.tile([128, 4 * D], dt, tag="wrhs")
                for dc in range(4):
                    nc.sync.dma_start(out=t[:, dc * D:(dc + 1) * D],
                                      in_=dram[dc * 128:(dc + 1) * 128, :])
                return t

            def mm_rows(out_ap, lhsT_dram, tix, w_sb, ncol, bias_sb, evac="v",
                        ldt=F16, ltag="lhst"):
                """out[128 rows of tile tix, ncol] = lhsT_dram[:, tile].T @ W (+ bias)."""
                ps = psp.tile([128, 512], F32, space="PSUM", tag="mmps")
                have_bias = bias_sb is not None
                if have_bias:
                    nc.tensor.matmul(ps[:, :ncol], ones1[:1, :128],
                                     bias_sb[:1, :ncol], start=True, stop=False)
                for dc in range(4):
                    lh = ldp.tile([128, 128], ldt, tag=ltag)
                    nc.sync.dma_start(
                        out=lh[:], in_=lhsT_dram[dc * 128:(dc + 1) * 128,
                                                 tix * 128:(tix + 1) * 128])
                    nc.tensor.matmul(ps[:, :ncol], lh[:],
                                     w_sb[:, dc * ncol:(dc + 1) * ncol],
                                     start=(not have_bias and dc == 0),
                                     stop=(dc == 3))
                if evac == "v":
                    nc.vector.tensor_copy(out_ap, ps[:, :ncol])
                else:  # DRAM destination: stage through SBUF (DMA can't read PSUM)
                    stg = ldp.tile([128, 512], F16, tag="stgb")
                    nc.scalar.copy(stg[:, :ncol], ps[:, :ncol])
                    nc.sync.dma_start(out=out_ap, in_=stg[:, :ncol])

            Wt_t = load_w(Wt_d)
            for lt in range(NT):
                mm_rows(U2[lt][:, 0:D], tT, lt, Wt_t[:], D, bt_sb)
            WtR_t = load_w(WtR_d)
            for lt in range(NT):
                mm_rows(U2[lt][:, D:2 * D], tT, lt, WtR_t[:], D, btR_sb)

            Wtr3_sb = wp.tile([128, 4 * 48], F32, tag="wtr3")
            Wacl_sb = wp.tile([128, 4 * 32], F16, tag="wacl")
            for dc in range(4):
                sl = slice(dc * 128, (dc + 1) * 128)
                nc.sync.dma_start(out=Wtr3_sb[:, dc * 48:(dc + 1) * 48], in_=Wtr3_d[sl, :])
                nc.sync.dma_start(out=Wacl_sb[:, dc * 32:(dc + 1) * 32], in_=Wacl_d[sl, :])
            for lt in range(NT):
                mm_rows(trio[lt][:], tT32, lt, Wtr3_sb[:], 48, None,
                        ldt=F32, ltag="lhst32")

            Wc_t = load_w(Wc_d)
            for kt in range(KT):
                mm_rows(Tctx[kt * 128:(kt + 1) * 128, 0:D], cT, kt, Wc_t[:], D,
                        bc_sb, evac="dma")
                mm_rows(Tctx[kt * 128:(kt + 1) * 128, D:D + 32], cT, kt, Wacl_sb[:], 32,
                        bacl_sb, evac="dma")

            # tpbT: per-octet score bias columns, partition layout p = ls*16 + e
            for lt in range(NT):
                for oc in range(16):
                    nc.sync.dma_start(
                        out=tpbT[:, lt * 16 + oc:lt * 16 + oc + 1],
                        in_=trio[lt][oc * 8:(oc + 1) * 8, 0:EH])

            # cpT [16, K] = Ws1c.T @ context^T + bs1, then replicate 8x on partitions
            cpT_t = wp.tile([EH, K], F32, tag="cpTt")
            cpT = cpT_t[:]
            for nt4 in range(4):
                nsl = slice(nt4 * 512, (nt4 + 1) * 512)
                ps = psp.tile([128, 512], F32, space="PSUM", tag="mmps")
                nc.tensor.matmul(ps[:EH, :], bs1_sb[:1, :], ones1[:1, :512],
                                 start=True, stop=False)
                for dc in range(4):
                    lh = ldp.tile([128, EH], F32, tag="lhst16")
                    nc.sync.dma_start(out=lh[:],
                                      in_=Ws1c_d[dc * 128:(dc + 1) * 128, :])
                    rh = ldp.tile([128, 512], F32, tag="ctchunk")
                    nc.sync.dma_start(out=rh[:], in_=cT32[dc * 128:(dc + 1) * 128, nsl])
                    nc.tensor.matmul(ps[:EH, :], lh[:], rh[:],
                                     start=False, stop=(dc == 3))
                nc.vector.tensor_copy(cpT[:, nsl], ps[:EH, :])
            for ls in range(8):
                nc.sync.dma_start(out=cpTrep[ls * 16:(ls + 1) * 16, :], in_=cpT[:, :])

            # ---------- per l-tile: scores, topk, gather, edge prep ----------
            def scores_phase(lt):
                # psum chunks accumulate over 16 octet matmuls; stationary
                # Ws2oc[:, oc] is one-hot so octet oc lands on rows oc*8+ls.
                scores = scp.tile([128, K], F32, tag="scores")
                for hf in range(2):
                    pssc = psscp.tile([128, 1024], F32, space="PSUM", tag="scps")
                    for oc in range(16):
                        g_oc = glp.tile([128, 1024], F32, tag="goc")
                        nc.scalar.activation(
                            g_oc[:], cpTrep[:, hf * 1024:(hf + 1) * 1024], ACTF.Gelu,
                            bias=tpbT[:, lt * 16 + oc:lt * 16 + oc + 1])
                        for c in range(2):
                            nc.tensor.matmul(
                                pssc[:, c * 512:(c + 1) * 512],
                                Ws2oc_sb[:, oc * 128:(oc + 1) * 128],
                                g_oc[:, c * 512:(c + 1) * 512],
                                start=(oc == 0), stop=(oc == 15))
                    nc.scalar.copy(scores[:, hf * 1024:(hf + 1) * 1024], pssc[:, :])
                mx8 = smlp.tile([128, 8], F32, tag="mx8")
                idx = smlp.tile([128, 8], U32, tag="idx")
                nc.vector.max(out=mx8[:], in_=scores[:])
                nc.vector.max_index(out=idx[:], in_max=mx8[:], in_values=scores[:])

                # ----- gather context-side rows -----
                gb = gbp.tile([128, WWIN * CROW], F16, tag="gb")
                for w in range(WWIN):
                    nc.gpsimd.indirect_dma_start(
                        out=gb[:, w * CROW:(w + 1) * CROW],
                        out_offset=None,
                        in_=Tctx[:, :],
                        in_offset=bass.IndirectOffsetOnAxis(ap=idx[:, w:w + 1], axis=0),
                    )

                # ----- per-edge angles -> cos/sin (fp16, full 64-wide) -----
                jf = smlp.tile([128, 8], F32, tag="jf")
                nc.vector.tensor_copy(jf[:], idx[:])
                delta = smlp.tile([128, 8], F32, tag="delta")
                nc.vector.tensor_scalar(delta[:], jf[:], lcol_sb[:, lt:lt + 1],
                                        scalar2=None, op0=ALU.subtract)
                ang = angp.tile([128, 8 * HD], F32, tag="ang")
                nc.vector.tensor_tensor(
                    out=ang[:].rearrange("p (w f) -> p w f", w=8),
                    in0=delta[:].unsqueeze(2).to_broadcast((128, 8, HD)),
                    in1=invf_sb[:].unsqueeze(1).to_broadcast((128, 8, HD)),
                    op=ALU.mult)
                # range-reduce to [-pi, pi]: x - 2pi*round(x/2pi), round via
                # the +/- 1.5*2^23 magic-number trick (no mod/floor on DVE ISA)
                MAGIC = 1.5 * 2.0 ** 23
                angt = angp.tile([128, 8 * HD], F32, tag="angt")
                nc.vector.tensor_scalar_mul(angt[:], ang[:], 1.0 / TWO_PI)
                angr = angp.tile([128, 8 * HD], F32, tag="angr")
                nc.vector.tensor_scalar(angr[:], angt[:], MAGIC, scalar2=MAGIC,
                                        op0=ALU.add, op1=ALU.subtract)
                nc.vector.tensor_sub(angt[:], angt[:], angr[:])
                nc.vector.tensor_scalar_mul(ang[:], angt[:], TWO_PI)
                cs = medp.tile([128, 2 * 8 * HD], F16, tag="cs")  # [cos | sin]
                nc.scalar.activation(cs[:, 512:1024], ang[:], ACTF.Sin, scale=-1.0)
                nc.vector.tensor_scalar_mul(angr[:], ang[:], -1.0)
                nc.vector.tensor_max(angt[:], ang[:], angr[:])
                nc.scalar.activation(cs[:, 0:512], angt[:], ACTF.Sin, scale=-1.0,
                                     bias=halfpi[:, 0:1])

                # ----- alphas = softplus(gelu(ta+ca) @ Wa2 + ba2), as fp16 -----
                gbv = gb[:].rearrange("p (w c) -> p w c", w=8)
                ha = smlp.tile([128, 8 * EH], F16, tag="ha")
                nc.vector.tensor_tensor(
                    out=ha[:].rearrange("p (w c) -> p w c", w=8),
                    in0=trio[lt][:, 16:32].unsqueeze(1).to_broadcast((128, 8, EH)),
                    in1=gbv[:, :, D:D + EH],
                    op=ALU.add)
                ha2 = smlp.tile([128, 8 * EH], F16, tag="ha2")
                nc.scalar.activation(ha2[:], ha[:], ACTF.Gelu)
                haw = smlp.tile([128, 8 * EH], F32, tag="haw")
                nc.vector.tensor_tensor(
                    out=haw[:].rearrange("p (w c) -> p w c", w=8),
                    in0=ha2[:].rearrange("p (w c) -> p w c", w=8),
                    in1=wa2_sb[:].unsqueeze(1).to_broadcast((128, 8, EH)),
                    op=ALU.mult)
                alph0 = smlp.tile([128, 8], F32, tag="alph0")
                nc.vector.tensor_reduce(alph0[:], haw[:].rearrange(
                    "p (w c) -> p w c", w=8), axis=AX.X, op=ALU.add)
                alphas = smlp.tile([128, 8], F32, tag="alphas")
                softplus(alphas[:], alph0[:], ba2_sb[:, 0:1], smlp, "sptmp")
                alphf = smlp.tile([128, 8], F16, tag="alphf")
                nc.vector.tensor_copy(alphf[:], alphas[:])

                # ----- g = gelu(tl + cl) and per-w transposes -----
                gmat = smlp.tile([128, 8 * EH], F32, tag="gmat")
                nc.vector.tensor_tensor(
                    out=gmat[:].rearrange("p (w c) -> p w c", w=8),
                    in0=trio[lt][:, 32:48].unsqueeze(1).to_broadcast((128, 8, EH)),
                    in1=gbv[:, :, D + EH:D + 2 * EH],
                    op=ALU.add)
                nc.scalar.activation(gmat[:], gmat[:], ACTF.Gelu)
                gT4 = gbp.tile([128, 2 * 128], F16, tag="gT4")  # 2 quads side by side
                nc.vector.memset(gT4[:], 0.0)
                for s4 in range(4):  # bias row (constant 1) for the bl2 fold
                    nc.sync.dma_start(out=gT4[32 * s4 + EH:32 * s4 + EH + 1, :],
                                      in_=onesf[:1, 0:256])
                for w in range(WWIN):
                    q, s = w // 4, w % 4
                    pst = psp.tile([EH, 128], F32, space="PSUM", tag="small")
                    nc.tensor.transpose(
                        out=pst[:, :],
                        in_=gmat[:].rearrange("p (w c) -> p w c", w=8)[:, w, :],
                        identity=ident)
                    nc.vector.tensor_copy(
                        gT4[32 * s:32 * s + EH, q * 128:(q + 1) * 128], pst[:, :])
                return idx, cs, gb, alphf, gT4

            def head_phase(lt, h, cs, gb, alphf, gT4):
                """One head of one l-tile: Lam matmuls + T iteration steps."""
                # ----- LamA (w, r, d) fp16 via PE; ACT evacuates psum pairs -----
                LamA = lamAp.tile([128, WRD], F16, tag="lamA")
                for wp_ in range(4):
                    psA = pslap.tile([128, 1024], F32, space="PSUM", tag="lamps")
                    for j in range(2):
                        w = wp_ * 2 + j
                        s, q2 = w % 4, w // 4
                        nc.tensor.matmul(
                            psA[:, j * 512:(j + 1) * 512],
                            gT4[32 * s:32 * s + 32, q2 * 128:(q2 + 1) * 128],
                            Wl2_sb[32 * s:32 * s + 32, h * R * HD:(h + 1) * R * HD],
                            start=True, stop=True, tile_position=(32 * s, 0))
                    nc.scalar.copy(
                        LamA[:, wp_ * 2 * R * HD:(wp_ * 2 + 2) * R * HD], psA[:, :])

                # ----- norms: n2 = sum_d LamA^2; rec = 1/max(n2, eps) -----
                sq = prodp.tile([128, DWR], F16, tag="scr")
                nc.vector.tensor_tensor(sq[:, 0:WRD], LamA[:], LamA[:], op=ALU.mult)
                n2 = smlp.tile([128, WWIN * R], F32, tag="n2")
                nc.vector.tensor_reduce(
                    n2[:].rearrange("p (g r) -> p g r", r=R),
                    sq[:, 0:WRD].rearrange("p (g r d) -> p g r d", r=R, d=HD),
                    axis=AX.X, op=ALU.add)
                nc.vector.tensor_scalar_max(n2[:], n2[:], 1e-24)
                rec = smlp.tile([128, WWIN * R], F32, tag="rec")
                nc.vector.reciprocal(rec[:], n2[:])

                # ----- LamB (d, w, r9) via one strided GpSimd copy -----
                LamB = lamBp.tile([128, DWR], F16, tag="lamB")
                lamB4 = LamB[:].rearrange("p (d w r) -> p d w r", d=HD, w=WWIN)
                nc.gpsimd.tensor_copy(
                    lamB4[:, :, :, 0:R],
                    LamA[:].rearrange("p (w r d) -> p w r d", w=WWIN, r=R)
                    .transpose([0, 3, 1, 2]))

                # ld2 slot 8 = alpha (the per-t mult rewrites only slots 0..7)
                ld2f = smlp.tile([128, WWIN * R9], F16, tag="ld2f")
                nc.vector.tensor_copy(
                    ld2f[:].rearrange("p (w r) -> p w r", r=R9)[:, :, 8:9].squeeze(2),
                    alphf[:])

                usl = U2[lt][:, h * HD:(h + 1) * HD]
                uRsl = U2[lt][:, D + h * HD:D + (h + 1) * HD]
                for t in range(T):
                    stc = slice(t * NT + lt, t * NT + lt + 1)
                    # fp16 shadow of [u | uR] for this head
                    ufp = medp.tile([128, 2 * HD], F16, tag="ufp")
                    nc.vector.tensor_copy(
                        ufp[:].rearrange("p (c x) -> p c x", c=2),
                        U2[lt][:].rearrange("p (c x) -> p c x", c=2)
                        [:, :, h * HD:(h + 1) * HD])
                    # diff = u*cos + uR*sin - v      [w, d] fp16
                    t0 = scrp.tile([128, WWIN * HD], F16, tag="t0")
                    nc.vector.tensor_tensor(
                        out=t0[:].rearrange("p (w d) -> p w d", w=WWIN),
                        in0=ufp[:, 0:HD].unsqueeze(1).to_broadcast((128, WWIN, HD)),
                        in1=cs[:, 0:512].rearrange("p (w d) -> p w d", w=WWIN),
                        op=ALU.mult)
                    t1 = scrp.tile([128, WWIN * HD], F16, tag="t1")
                    nc.vector.tensor_tensor(
                        out=t1[:].rearrange("p (w d) -> p w d", w=WWIN),
                        in0=ufp[:, HD:].unsqueeze(1).to_broadcast((128, WWIN, HD)),
                        in1=cs[:, 512:1024].rearrange("p (w d) -> p w d", w=WWIN),
                        op=ALU.mult)
                    nc.vector.tensor_tensor(t0[:], t0[:], t1[:], op=ALU.add)
                    diff = scrp.tile([128, WWIN * HD], F16, tag="diff")
                    nc.vector.tensor_tensor(
                        out=diff[:].rearrange("p (w d) -> p w d", w=WWIN),
                        in0=t0[:].rearrange("p (w d) -> p w d", w=WWIN),
                        in1=gb[:].rearrange("p (w c) -> p w c", w=8)
                            [:, :, h * HD:(h + 1) * HD],
                        op=ALU.subtract)
                    # diff into LamB slot r=8 (strided; GpSimd keeps DVE free)
                    nc.gpsimd.tensor_copy(
                        lamB4[:, :, :, 8:9].squeeze(3).transpose([0, 2, 1]),
                        diff[:].rearrange("p (w d) -> p w d", w=WWIN))
                    # einsum1: ld[w,r] = sum_d LamA*diff
                    prod = prodp.tile([128, DWR], F16, tag="scr")
                    p1 = prod[:, 0:WRD].rearrange("p (g r d) -> p g r d", r=R, d=HD)
                    nc.vector.tensor_tensor(
                        out=p1,
                        in0=LamA[:].rearrange("p (g r d) -> p g r d", r=R, d=HD),
                        in1=diff[:].rearrange("p (g d) -> p g d", d=HD)
                            .unsqueeze(2).to_broadcast((128, WWIN, R, HD)),
                        op=ALU.mult)
                    ld8 = smlp.tile([128, WWIN * R], F32, tag="ld8")
                    nc.vector.tensor_reduce(
                        ld8[:].rearrange("p (g r) -> p g r", r=R), p1,
                        axis=AX.X, op=ALU.add)
                    # ld2 slots 0..8 = ld * rec (fp16 feeds the packed prod2)
                    nc.vector.tensor_tensor(
                        ld2f[:].rearrange("p (w r) -> p w r", r=R9)[:, :, 0:R],
                        ld8[:].rearrange("p (w r) -> p w r", r=R),
                        rec[:].rearrange("p (w r) -> p w r", r=R),
                        op=ALU.mult)
                    # einsum2 + alpha slot + scatter: rsum[d] = sum over m=(w,r9)
                    nc.vector.tensor_tensor(
                        out=prod[:].rearrange("p (d m) -> p d m", d=HD),
                        in0=LamB[:].rearrange("p (d m) -> p d m", d=HD),
                        in1=ld2f[:].unsqueeze(1).to_broadcast(
                            (128, HD, WWIN * R9)),
                        op=ALU.mult)
                    rsum = smlp.tile([128, HD], F32, tag="rsum")
                    nc.vector.tensor_reduce(
                        rsum[:], prod[:].rearrange("p (d m) -> p d m", d=HD),
                        axis=AX.X, op=ALU.add)
                    # u -= step*rsum ; uR via rotate_half identity
                    nc.vector.scalar_tensor_tensor(
                        out=usl, in0=rsum[:], scalar=stpn_sb[:, stc],
                        in1=usl, op0=ALU.mult, op1=ALU.add)
                    nc.vector.scalar_tensor_tensor(
                        out=uRsl[:, 0:32], in0=rsum[:, 32:64],
                        scalar=stp_sb[:, stc], in1=uRsl[:, 0:32],
                        op0=ALU.mult, op1=ALU.add)
                    nc.vector.scalar_tensor_tensor(
                        out=uRsl[:, 32:64], in0=rsum[:, 0:32],
                        scalar=stpn_sb[:, stc], in1=uRsl[:, 32:64],
                        op0=ALU.mult, op1=ALU.add)

            for lt in range(NT):
                idx, cs, gb, alphf, gT4 = scores_phase(lt)
                for h in range(H):
                    head_phase(lt, h, cs, gb, alphf, gT4)

            # ---------- output projection: y = u @ Wo + bo ----------
            Wo_t = load_w(Wo_d)
            for lt in range(NT):
                psy = psp.tile([128, 512], F32, space="PSUM", tag="mmps")
                nc.tensor.matmul(psy[:, :], ones1[:1, :128], bo_sb[:1, :],
                                 start=True, stop=False)
                for dc in range(4):
                    pst = psp.tile([128, 128], F32, space="PSUM", tag="small")
                    nc.tensor.transpose(
                        out=pst[:, :], in_=U2[lt][:, dc * 128:(dc + 1) * 128],
                        identity=ident)
                    uT = ldp.tile([128, 128], F16, tag="uT")
                    nc.vector.tensor_copy(uT[:], pst[:, :])
                    nc.tensor.matmul(psy[:, :], uT[:], Wo_t[:, dc * D:(dc + 1) * D],
                                     start=False, stop=(dc == 3))
                ystg = ldp.tile([128, 512], F32, tag="stg")
                nc.vector.tensor_copy(ystg[:], psy[:, :])
                nc.sync.dma_start(out=y_d[lt * 128:(lt + 1) * 128, :], in_=ystg[:])

    nc.finalize()
    return nc


def _rot_cols(Wm):
    """Fold rotate_half into output columns: out cols = [-cols(h, hi), cols(h, lo)]."""
    W4 = Wm.reshape(-1, H, 2, HD // 2)
    out = np.concatenate([-W4[:, :, 1], W4[:, :, 0]], axis=2)
    return np.ascontiguousarray(out.reshape(Wm.shape))


def make_in_maps(inputs):
    """Host-side prep: slice/transpose inputs into the 8 per-core input maps."""
    f16 = np.float16
    target = np.asarray(inputs["target"], np.float32)
    context = np.asarray(inputs["context"], np.float32)
    Wt = np.asarray(inputs["Wt"], np.float32)
    bt = np.asarray(inputs["bt"], np.float32)
    Wc = np.asarray(inputs["Wc"], np.float32)
    bc = np.asarray(inputs["bc"], np.float32)
    Ws1 = np.asarray(inputs["Ws1"], np.float32)
    bs1 = np.asarray(inputs["bs1"], np.float32)
    Ws2 = np.asarray(inputs["Ws2"], np.float32)
    Wa1 = np.asarray(inputs["Wa1"], np.float32)
    ba1 = np.asarray(inputs["ba1"], np.float32)
    Wa2 = np.asarray(inputs["Wa2"], np.float32)
    ba2 = np.asarray(inputs["ba2"], np.float32)
    Wl1 = np.asarray(inputs["Wl1"], np.float32)
    bl1 = np.asarray(inputs["bl1"], np.float32)
    Wl2 = np.asarray(inputs["Wl2"], np.float32)
    bl2 = np.asarray(inputs["bl2"], np.float32)
    step_sizes = np.asarray(inputs["step_sizes"], np.float32)
    Wo = np.asarray(inputs["Wo"], np.float32)
    bo = np.asarray(inputs["bo"], np.float32)

    # per-octet one-hot stationary: col block oc, Ws2oc[ls*16+e, oc*8+ls]=Ws2[e]
    Ws2oc = np.zeros((128, 16 * 128), np.float32)
    for oc in range(16):
        for ls in range(8):
            Ws2oc[ls * 16:(ls + 1) * 16, oc * 128 + oc * 8 + ls] = Ws2[:, 0]
    Wl2s = np.zeros((128, H * R * HD), np.float32)
    for s in range(4):
        Wl2s[32 * s:32 * s + EH, :] = Wl2
        Wl2s[32 * s + EH, :] = bl2
    invf32 = 1.0 / (10000.0 ** (np.arange(0, HD, 2, dtype=np.float32) / HD))
    invf = np.concatenate([invf32, invf32])[None, :]

    common = dict(
        Wt=Wt.astype(f16), WtR=_rot_cols(Wt).astype(f16),
        Wc=Wc.astype(f16), Wo=Wo.astype(f16),
        bt=bt[None, :], btR=_rot_cols(bt[None, :]), bc=bc[None, :], bo=bo[None, :],
        Wtr3=np.ascontiguousarray(
            np.concatenate([Ws1[:D], Wa1[:D], Wl1[:D]], axis=1)),
        Ws1c=np.ascontiguousarray(Ws1[D:]),
        Wacl=np.ascontiguousarray(
            np.concatenate([Wa1[D:], Wl1[D:]], axis=1)).astype(f16),
        bs1=bs1[None, :],
        bacl=np.concatenate([ba1, bl1])[None, :],
        Ws2oc=Ws2oc, Wa2=np.ascontiguousarray(Wa2.T),
        ba2=np.asarray(ba2, np.float32).reshape(1, 1), Wl2=Wl2s.astype(f16),
        invf=np.ascontiguousarray(invf, np.float32),
    )

    in_maps = []
    for c in range(8):
        b, rc = c // 4, c % 4
        rows = slice(rc * LC, (rc + 1) * LC)
        stp = np.ascontiguousarray(
            step_sizes[:, rows].reshape(T, NT, 128).transpose(2, 0, 1)
            .reshape(128, T * NT))
        lcol = np.ascontiguousarray(
            (rc * LC + np.arange(LC, dtype=np.float32)).reshape(NT, 128).T)
        tT32 = np.ascontiguousarray(target[b, rows].T)
        cT32 = np.ascontiguousarray(context[b].T)
        m = dict(common)
        m.update(
            tT=tT32.astype(f16), tT32=tT32,
            cT=cT32.astype(f16), cT32=cT32,
            stp=stp, lcol=lcol,
        )
        in_maps.append(m)
    return in_maps


_NC_CACHE = {}


def kernel(**inputs):
    if "nc" not in _NC_CACHE:
        _NC_CACHE["nc"] = build_program()
    nc = _NC_CACHE["nc"]
    in_maps = make_in_maps(inputs)
    res = run_bass_kernel_spmd(nc, in_maps, list(range(8)))
    out = np.empty((B, L, D), np.float32)
    for c in range(8):
        b, rc = c // 4, c % 4
        out[b, rc * LC:(rc + 1) * LC] = res.results[c]["y"]
    return out
